# revision 28
# baseline (speedup 1.0000x reference)
"""Trainium2 Bass kernel for BlockDecomposedSSMAttention.

Math: y[b,s,:] = x[b,s,:] @ B.T @ A @ C.T   (no cross-block recurrence)
 ==>  y = x @ W  with  W[i,o] = sum_{h,k} B[h,i] A[h,k] C[o,k]

Distribution over the 8 NeuronCores: 2D grid, RG row-groups x G col-groups.
  - rows (batch*seq = 16384) split across RG groups,
  - output columns (1024) split across G groups of width W_COL = 1024/G.
Each core computes ONLY its W[:, col-slice] (cost scales with W_COL: the
param chain is folded C-slice-first: M1 = A @ C_slice.T, then
W_slice = B.T @ M1), so widening the grid in G trades redundant W work
for extra x DMA (each x row is read by G cores). With G=4:
  PE cycles/core = 2*8*8*W_COL (stages) + R*8*W_COL/... = 32768 + 131072
                 = 163840  (vs 262144 for the replicated-W baseline)
  DMA-in/core    = 16MB x (bf16) + 4.5MB params = 20.5MB  (~57us @ 360GB/s)
All device data is bf16 (PSUM accumulation fp32): halves DMA vs f32r at
~4e-3 scale-relative absmax (gate is 2e-2). The main loop is transposed
(out = [o_part, m_moving]) so its moving dim stays 512 even at W_COL=256;
y comes back [W_COL, R] and the host transposes it.

Host-side work is layout marshalling + dtype casts only; every FLOP runs
on the device.
"""

import os
import sys

import numpy as np

if "/opt/trn_rl_repo" not in sys.path:
    sys.path.insert(0, "/opt/trn_rl_repo")

import ml_dtypes

BF16 = ml_dtypes.bfloat16

BATCH, SEQ, D = 4, 4096, 1024
NCORES = 8
ROWS = BATCH * SEQ            # 16384
P = 128
KT = D // P                   # 8 tiles along any 1024 dim

G = int(os.environ.get("BASS_G", "4"))      # column groups
RG = NCORES // G                            # row groups
R = ROWS // RG                              # rows per core
W_COL = D // G                              # output cols per core
MC = R // 512                               # main-loop m chunks (512 wide)
OT = W_COL // P                             # main-loop o tiles

_CACHE: dict = {}


def _build_nc():
    import concourse.mybir as mybir
    import concourse.tile as tile
    from concourse import bacc

    f32 = mybir.dt.float32
    bf16 = mybir.dt.bfloat16

    nc = bacc.Bacc(
        "TRN2", target_bir_lowering=False, debug=False, num_devices=NCORES
    )

    # Per-core DRAM I/O, all bf16, laid out so every SBUF load is a
    # contiguous-per-partition chunk and every matmul operand has the
    # contraction dim on partitions.
    ct_d = nc.dram_tensor("ct_d", [P, KT, W_COL], bf16, kind="ExternalInput")
    at_d = nc.dram_tensor("at_d", [P, KT, KT, P], bf16, kind="ExternalInput")
    b_d = nc.dram_tensor("b_d", [P, KT, KT, P], bf16, kind="ExternalInput")
    xt_d = nc.dram_tensor("xt_d", [P, MC, KT, 512], bf16, kind="ExternalInput")
    y_d = nc.dram_tensor("y_d", [W_COL, R], bf16, kind="ExternalOutput")

    with tile.TileContext(nc) as tc:
        with (
            tc.tile_pool(name="psw", bufs=2, space="PSUM") as psw,
            tc.tile_pool(name="psm", bufs=3, space="PSUM") as psm,
            tc.tile_pool(name="ycopy", bufs=6) as ycopy,
        ):
            ct_sb, _f1 = tc.tile([P, KT, W_COL], bf16, name="ct_sb")
            at_sb, _f2 = tc.tile([P, KT, KT, P], bf16, name="at_sb")
            b_sb, _f3 = tc.tile([P, KT, KT, P], bf16, name="b_sb")
            m1_sb, _f4 = tc.tile([P, KT, W_COL], bf16, name="m1_sb")
            w_sb, _f5 = tc.tile([P, KT, W_COL], bf16, name="w_sb")
            xt_sb, _f6 = tc.tile([P, MC, KT, 512], bf16, name="xt_sb")
            # (PE warm-up via dummy matmuls was tried twice - vector-memset
            # and gpsimd-memset gated - and measured SLOWER both times (+1us,
            # +7us): the dummies sit ahead of stage 1 in PE program order and
            # any late memset/schedule slip delays real work. Don't warm up.)

            # ---- DMA issue ----
            # One strictly-ordered load stream on sync (SP HWDGE), in
            # consumption order: parallel streams on other engines would
            # interleave on the shared DMA queues and starve the
            # critical-path params (b arriving late stalls stage 2).
            # gpsimd (SWDGE) carries the y stores (issued in main loop).
            # (first chunks on the scalar HWDGE measured ~3us WORSE: the Act
            # engine's first DMA config issues ~1us later than sync's, so the
            # "parallel" path delayed the first matmul instead)
            nc.sync.dma_start(ct_sb[:, 0:4], ct_d.ap()[:, 0:4])
            nc.sync.dma_start(at_sb[:, 0, 0:4], at_d.ap()[:, 0, 0:4])
            nc.sync.dma_start(ct_sb[:, 4:KT], ct_d.ap()[:, 4:KT])
            nc.sync.dma_start(at_sb[:, 0, 4:KT], at_d.ap()[:, 0, 4:KT])
            for t in range(1, KT):
                nc.sync.dma_start(at_sb[:, t], at_d.ap()[:, t])
            # b is laid out + loaded in stage-2 consumption order (i-tile
            # chunks): group t of stage 2 then waits only on its own 0.25MB
            # chunk instead of the last byte of B, streaming like stage 1.
            for t in range(KT):
                nc.sync.dma_start(b_sb[:, t], b_d.ap()[:, t])
            for mc in range(MC):
                nc.sync.dma_start(xt_sb[:, mc], xt_d.ap()[:, mc])

            # ---- stage 1: M1[h,o] = sum_k A[h,k] C[o,k] ----
            for t in range(KT):
                p1 = psw.tile([P, W_COL], f32, tag="ps", name="p1")
                for j in range(KT):
                    nc.tensor.matmul(
                        p1[:],
                        at_sb[:, t, j, :],
                        ct_sb[:, j, :],
                        start=(j == 0),
                        stop=(j == KT - 1),
                    )
                nc.vector.tensor_copy(m1_sb[:, t, :], p1[:])

            # ---- stage 2: W[i,o] = sum_h B[h,i] M1[h,o] ----
            for t in range(KT):
                p2 = psw.tile([P, W_COL], f32, tag="ps", name="p2")
                for j in range(KT):
                    nc.tensor.matmul(
                        p2[:],
                        b_sb[:, t, j, :],
                        m1_sb[:, j, :],
                        start=(j == 0),
                        stop=(j == KT - 1),
                    )
                nc.vector.tensor_copy(w_sb[:, t, :], p2[:])

            # ---- main (transposed): yT[o,m] = sum_i W[i,o] x[m,i] ----
            # both o-tiles of an m-chunk accumulate into one 2-bank PSUM
            # tile, then drain with ONE copy + ONE store (halves DVE/DMA
            # instruction + semaphore traffic).  The last chunk keeps the
            # fine-grained per-o-tile path on sync HWDGE for the short tail.
            for mc in range(MC):
                if mc < MC - 1:
                    pm2 = psm.tile([P, OT, 512], f32, tag="pm", name="pm2")
                    for ot in range(OT):
                        for j in range(KT):
                            nc.tensor.matmul(
                                pm2[:, ot, :],
                                w_sb[:, j, ot * P : (ot + 1) * P],
                                xt_sb[:, mc, j, :],
                                start=(j == 0),
                                stop=(j == KT - 1),
                            )
                    yt = ycopy.tile([P, OT, 512], bf16, tag="yt", name="yt")
                    nc.vector.tensor_copy(yt[:], pm2[:])
                    nc.gpsimd.dma_start(
                        y_d.ap()[:, mc * 512 : (mc + 1) * 512].rearrange(
                            "(s p) m -> p s m", p=P
                        ),
                        yt[:],
                    )
                else:
                    for ot in range(OT):
                        pm2 = psm.tile([P, OT, 512], f32, tag="pm", name="pm2")
                        for j in range(KT):
                            nc.tensor.matmul(
                                pm2[:, 0, :],
                                w_sb[:, j, ot * P : (ot + 1) * P],
                                xt_sb[:, mc, j, :],
                                start=(j == 0),
                                stop=(j == KT - 1),
                            )
                        yt = ycopy.tile([P, OT, 512], bf16, tag="yt", name="yt")
                        nc.vector.tensor_copy(yt[:, 0, :], pm2[:, 0, :])
                        nc.sync.dma_start(
                            y_d.ap()[
                                ot * P : (ot + 1) * P, mc * 512 : (mc + 1) * 512
                            ],
                            yt[:, 0, :],
                        )

            for f in (_f6, _f5, _f4, _f3, _f2, _f1):
                f()

    nc.compile()
    return nc


def _get_nc():
    if "nc" not in _CACHE:
        _CACHE["nc"] = _build_nc()
    return _CACHE["nc"]


def _make_in_maps(x, A, B, C):
    x2 = np.ascontiguousarray(x, dtype=np.float32).reshape(ROWS, D).astype(BF16)
    a16 = np.asarray(A, dtype=np.float32).astype(BF16)
    b16 = np.asarray(B, dtype=np.float32).astype(BF16)
    c16 = np.asarray(C, dtype=np.float32).astype(BF16)

    # at_d[p,t,j,c] = A[t*128+c, j*128+p]
    at = np.ascontiguousarray(a16.reshape(KT, P, KT, P).transpose(3, 0, 2, 1))
    # b_d[p,t,j,c] = B[j*128+p, t*128+c]  (i-tile-major: stage-2 order)
    bm = np.ascontiguousarray(b16.reshape(KT, P, KT, P).transpose(1, 2, 0, 3))

    in_maps = []
    for c in range(NCORES):
        rg, cg = divmod(c, G)
        # ct_d[p,j,o] = C[cg*W_COL+o, j*128+p]
        ct = np.ascontiguousarray(
            c16[cg * W_COL : (cg + 1) * W_COL].reshape(W_COL, KT, P).transpose(2, 1, 0)
        )
        rows = x2[rg * R : (rg + 1) * R]
        # xt_d[p,mc,j,m] = rows[mc*512+m, j*128+p]
        xtc = np.ascontiguousarray(
            rows.reshape(MC, 512, KT, P).transpose(3, 0, 2, 1)
        )
        in_maps.append({"xt_d": xtc, "at_d": at, "b_d": bm, "ct_d": ct})
    return in_maps


def _install_ntff_hook():
    """The agent image's ``antenv`` lacks ``axon_hooks``; recreate it and
    register the ctypes-based NTFF profile hook (same as trn_boot's
    ``_ntff_profile_via_ctypes``) so ``trace=True`` yields exec_time_ns."""
    import contextlib
    import ctypes
    import types

    if "antenv.axon_hooks" in sys.modules:
        return True
    so_path = "/opt/axon/libaxon_pjrt.so"
    if not os.path.exists(so_path):
        return False
    lib = ctypes.CDLL(so_path)
    if not hasattr(lib, "axon_start_nrt_profile"):
        return False
    lib.axon_start_nrt_profile.argtypes = [
        ctypes.POINTER(ctypes.c_int64),
        ctypes.c_size_t,
    ]
    lib.axon_start_nrt_profile.restype = ctypes.c_int64
    lib.axon_stop_nrt_profile.argtypes = [ctypes.c_char_p]
    lib.axon_stop_nrt_profile.restype = ctypes.c_int64

    @contextlib.contextmanager
    def _hook(output_dir, device_ids):
        import jax

        jax.devices()
        if device_ids:
            ids = (ctypes.c_int64 * len(device_ids))(*device_ids)
            rc = lib.axon_start_nrt_profile(ids, len(device_ids))
        else:
            rc = lib.axon_start_nrt_profile(None, 0)
        if rc != 0:
            raise RuntimeError(f"axon_start_nrt_profile rc={rc}")
        try:
            yield
        finally:
            n = lib.axon_stop_nrt_profile(str(output_dir).encode())
            print(f"ntff profile: {n} file(s) written to {output_dir}")

    mod = types.ModuleType("antenv.axon_hooks")
    _state = {"hook": _hook}
    mod.set_axon_ntff_profile_hook = lambda h: _state.__setitem__("hook", h)
    mod.get_axon_ntff_profile_hook = lambda: _state["hook"]
    sys.modules["antenv.axon_hooks"] = mod
    import antenv

    antenv.axon_hooks = mod
    return True


def run(x, A, B, C, trace=False):
    """Run on hardware; returns (y_full, exec_time_ns_or_None)."""
    from concourse import bass_utils
    from concourse.bass_interp import get_hw_module

    if trace and not _install_ntff_hook():
        trace = False
    if trace:
        # upload_artifacts pushes the NEFF dir to a remote bucket; in this
        # sandbox that can fail AFTER a successful run, losing the results.
        # Degrade to the local path. (Only touches the tracing dev path.)
        if not getattr(bass_utils.upload_artifacts, "_safe", False):
            _orig_upload = bass_utils.upload_artifacts

            def _safe_upload(tmpdir):
                try:
                    return _orig_upload(tmpdir)
                except Exception as e:
                    print(f"upload_artifacts skipped ({type(e).__name__}): {e}")
                    return str(tmpdir)

            _safe_upload._safe = True
            bass_utils.upload_artifacts = _safe_upload

    nc = _get_nc()
    in_maps = _make_in_maps(x, A, B, C)

    old_m = nc.m
    nc.m = get_hw_module(nc.m)
    try:
        res = bass_utils.run_bass_kernel_spmd(
            nc, in_maps, core_ids=list(range(NCORES)), trace=trace
        )
    finally:
        nc.m = old_m

    y = np.empty((ROWS, D), dtype=np.float32)
    for c in range(NCORES):
        rg, cg = divmod(c, G)
        yp = np.asarray(res.results[c]["y_d"]).astype(np.float32)  # [W_COL, R]
        y[rg * R : (rg + 1) * R, cg * W_COL : (cg + 1) * W_COL] = yp.T
    return y.reshape(BATCH, SEQ, D), res.exec_time_ns


def kernel(x, A, B, C):
    y, _ = run(x, A, B, C, trace=False)
    return y


# revision 31
# speedup vs baseline: 1.0037x; 1.0037x over previous
"""Trainium2 Bass kernel for BlockDecomposedSSMAttention.

Math: y[b,s,:] = x[b,s,:] @ B.T @ A @ C.T   (no cross-block recurrence)
 ==>  y = x @ W  with  W[i,o] = sum_{h,k} B[h,i] A[h,k] C[o,k]

Distribution over the 8 NeuronCores: 2D grid, RG row-groups x G col-groups.
  - rows (batch*seq = 16384) split across RG groups,
  - output columns (1024) split across G groups of width W_COL = 1024/G.
Each core computes ONLY its W[:, col-slice] (cost scales with W_COL: the
param chain is folded C-slice-first: M1 = A @ C_slice.T, then
W_slice = B.T @ M1), so widening the grid in G trades redundant W work
for extra x DMA (each x row is read by G cores). With G=4:
  PE cycles/core = 2*8*8*W_COL (stages) + R*8*W_COL/... = 32768 + 131072
                 = 163840  (vs 262144 for the replicated-W baseline)
  DMA-in/core    = 16MB x (bf16) + 4.5MB params = 20.5MB  (~57us @ 360GB/s)
All device data is bf16 (PSUM accumulation fp32): halves DMA vs f32r at
~4e-3 scale-relative absmax (gate is 2e-2). The main loop is transposed
(out = [o_part, m_moving]) so its moving dim stays 512 even at W_COL=256;
y comes back [W_COL, R] and the host transposes it.

Host-side work is layout marshalling + dtype casts only; every FLOP runs
on the device.
"""

import os
import sys

import numpy as np

if "/opt/trn_rl_repo" not in sys.path:
    sys.path.insert(0, "/opt/trn_rl_repo")

import ml_dtypes

BF16 = ml_dtypes.bfloat16

BATCH, SEQ, D = 4, 4096, 1024
NCORES = 8
ROWS = BATCH * SEQ            # 16384
P = 128
KT = D // P                   # 8 tiles along any 1024 dim

G = int(os.environ.get("BASS_G", "4"))      # column groups
RG = NCORES // G                            # row groups
R = ROWS // RG                              # rows per core
W_COL = D // G                              # output cols per core
MC = R // 512                               # main-loop m chunks (512 wide)
OT = W_COL // P                             # main-loop o tiles

_CACHE: dict = {}


def _build_nc():
    import concourse.mybir as mybir
    import concourse.tile as tile
    from concourse import bacc

    f32 = mybir.dt.float32
    bf16 = mybir.dt.bfloat16

    nc = bacc.Bacc(
        "TRN2", target_bir_lowering=False, debug=False, num_devices=NCORES
    )

    # Per-core DRAM I/O, all bf16, laid out so every SBUF load is a
    # contiguous-per-partition chunk and every matmul operand has the
    # contraction dim on partitions.
    ct_d = nc.dram_tensor("ct_d", [P, KT, W_COL], bf16, kind="ExternalInput")
    at_d = nc.dram_tensor("at_d", [P, KT, KT, P], bf16, kind="ExternalInput")
    b_d = nc.dram_tensor("b_d", [P, KT, KT, P], bf16, kind="ExternalInput")
    xt_d = nc.dram_tensor("xt_d", [P, MC, KT, 512], bf16, kind="ExternalInput")
    y_d = nc.dram_tensor("y_d", [W_COL, R], bf16, kind="ExternalOutput")

    with tile.TileContext(nc) as tc:
        with (
            tc.tile_pool(name="psw", bufs=2, space="PSUM") as psw,
            tc.tile_pool(name="psm", bufs=3, space="PSUM") as psm,
            tc.tile_pool(name="ycopy", bufs=6) as ycopy,
        ):
            ct_sb, _f1 = tc.tile([P, KT, W_COL], bf16, name="ct_sb")
            at_sb, _f2 = tc.tile([P, KT, KT, P], bf16, name="at_sb")
            b_sb, _f3 = tc.tile([P, KT, KT, P], bf16, name="b_sb")
            m1_sb, _f4 = tc.tile([P, KT, W_COL], bf16, name="m1_sb")
            w_sb, _f5 = tc.tile([P, KT, W_COL], bf16, name="w_sb")
            xt_sb, _f6 = tc.tile([P, MC, KT, 512], bf16, name="xt_sb")
            # (PE warm-up via dummy matmuls was tried twice - vector-memset
            # and gpsimd-memset gated - and measured SLOWER both times (+1us,
            # +7us): the dummies sit ahead of stage 1 in PE program order and
            # any late memset/schedule slip delays real work. Don't warm up.)

            # ---- DMA issue ----
            # One strictly-ordered load stream on sync (SP HWDGE), in
            # consumption order: parallel streams on other engines would
            # interleave on the shared DMA queues and starve the
            # critical-path params (b arriving late stalls stage 2).
            # gpsimd (SWDGE) carries the y stores (issued in main loop).
            # (first chunks on the scalar HWDGE measured ~3us WORSE: the Act
            # engine's first DMA config issues ~1us later than sync's, so the
            # "parallel" path delayed the first matmul instead)
            nc.sync.dma_start(ct_sb[:, 0:4], ct_d.ap()[:, 0:4])
            nc.sync.dma_start(at_sb[:, 0, 0:4], at_d.ap()[:, 0, 0:4])
            nc.sync.dma_start(ct_sb[:, 4:KT], ct_d.ap()[:, 4:KT])
            nc.sync.dma_start(at_sb[:, 0, 4:KT], at_d.ap()[:, 0, 4:KT])
            for t in range(1, KT):
                nc.sync.dma_start(at_sb[:, t], at_d.ap()[:, t])
            # b is laid out + loaded in stage-2 consumption order (i-tile
            # chunks): group t of stage 2 then waits only on its own 0.25MB
            # chunk instead of the last byte of B, streaming like stage 1.
            for t in range(KT):
                nc.sync.dma_start(b_sb[:, t], b_d.ap()[:, t])
            for mc in range(MC):
                nc.sync.dma_start(xt_sb[:, mc], xt_d.ap()[:, mc])

            # ---- stage 1: M1[h,o] = sum_k A[h,k] C[o,k] ----
            # two 256-wide groups fit exactly one PSUM bank: pair them and
            # drain with one copy (same trick as the main loop)
            for t2 in range(0, KT, 2):
                p1 = psw.tile([P, 2, W_COL], f32, tag="ps", name="p1")
                for s in range(2):
                    for j in range(KT):
                        nc.tensor.matmul(
                            p1[:, s, :],
                            at_sb[:, t2 + s, j, :],
                            ct_sb[:, j, :],
                            start=(j == 0),
                            stop=(j == KT - 1),
                        )
                nc.vector.tensor_copy(m1_sb[:, t2 : t2 + 2, :], p1[:])

            # ---- stage 2: W[i,o] = sum_h B[h,i] M1[h,o] ----
            for t2 in range(0, KT, 2):
                p2 = psw.tile([P, 2, W_COL], f32, tag="ps", name="p2")
                for s in range(2):
                    for j in range(KT):
                        nc.tensor.matmul(
                            p2[:, s, :],
                            b_sb[:, t2 + s, j, :],
                            m1_sb[:, j, :],
                            start=(j == 0),
                            stop=(j == KT - 1),
                        )
                nc.vector.tensor_copy(w_sb[:, t2 : t2 + 2, :], p2[:])

            # ---- main (transposed): yT[o,m] = sum_i W[i,o] x[m,i] ----
            # both o-tiles of an m-chunk accumulate into one 2-bank PSUM
            # tile, then drain with ONE copy + ONE store (halves DVE/DMA
            # instruction + semaphore traffic).  The last chunk keeps the
            # fine-grained per-o-tile path on sync HWDGE for the short tail.
            for mc in range(MC):
                if mc < MC - 1:
                    pm2 = psm.tile([P, OT, 512], f32, tag="pm", name="pm2")
                    for ot in range(OT):
                        for j in range(KT):
                            nc.tensor.matmul(
                                pm2[:, ot, :],
                                w_sb[:, j, ot * P : (ot + 1) * P],
                                xt_sb[:, mc, j, :],
                                start=(j == 0),
                                stop=(j == KT - 1),
                            )
                    yt = ycopy.tile([P, OT, 512], bf16, tag="yt", name="yt")
                    nc.vector.tensor_copy(yt[:], pm2[:])
                    nc.gpsimd.dma_start(
                        y_d.ap()[:, mc * 512 : (mc + 1) * 512].rearrange(
                            "(s p) m -> p s m", p=P
                        ),
                        yt[:],
                    )
                else:
                    for ot in range(OT):
                        pm2 = psm.tile([P, OT, 512], f32, tag="pm", name="pm2")
                        for j in range(KT):
                            nc.tensor.matmul(
                                pm2[:, 0, :],
                                w_sb[:, j, ot * P : (ot + 1) * P],
                                xt_sb[:, mc, j, :],
                                start=(j == 0),
                                stop=(j == KT - 1),
                            )
                        yt = ycopy.tile([P, OT, 512], bf16, tag="yt", name="yt")
                        # halve the copy->store serial latency on the tail:
                        # each half-copy's store fires as soon as it lands
                        for h in range(2):
                            nc.vector.tensor_copy(
                                yt[:, 0, h * 256 : (h + 1) * 256],
                                pm2[:, 0, h * 256 : (h + 1) * 256],
                            )
                            nc.sync.dma_start(
                                y_d.ap()[
                                    ot * P : (ot + 1) * P,
                                    mc * 512 + h * 256 : mc * 512 + (h + 1) * 256,
                                ],
                                yt[:, 0, h * 256 : (h + 1) * 256],
                            )

            for f in (_f6, _f5, _f4, _f3, _f2, _f1):
                f()

    nc.compile()
    return nc


def _get_nc():
    if "nc" not in _CACHE:
        _CACHE["nc"] = _build_nc()
    return _CACHE["nc"]


def _make_in_maps(x, A, B, C):
    x2 = np.ascontiguousarray(x, dtype=np.float32).reshape(ROWS, D).astype(BF16)
    a16 = np.asarray(A, dtype=np.float32).astype(BF16)
    b16 = np.asarray(B, dtype=np.float32).astype(BF16)
    c16 = np.asarray(C, dtype=np.float32).astype(BF16)

    # at_d[p,t,j,c] = A[t*128+c, j*128+p]
    at = np.ascontiguousarray(a16.reshape(KT, P, KT, P).transpose(3, 0, 2, 1))
    # b_d[p,t,j,c] = B[j*128+p, t*128+c]  (i-tile-major: stage-2 order)
    bm = np.ascontiguousarray(b16.reshape(KT, P, KT, P).transpose(1, 2, 0, 3))

    in_maps = []
    for c in range(NCORES):
        rg, cg = divmod(c, G)
        # ct_d[p,j,o] = C[cg*W_COL+o, j*128+p]
        ct = np.ascontiguousarray(
            c16[cg * W_COL : (cg + 1) * W_COL].reshape(W_COL, KT, P).transpose(2, 1, 0)
        )
        rows = x2[rg * R : (rg + 1) * R]
        # xt_d[p,mc,j,m] = rows[mc*512+m, j*128+p]
        xtc = np.ascontiguousarray(
            rows.reshape(MC, 512, KT, P).transpose(3, 0, 2, 1)
        )
        in_maps.append({"xt_d": xtc, "at_d": at, "b_d": bm, "ct_d": ct})
    return in_maps


def _install_ntff_hook():
    """The agent image's ``antenv`` lacks ``axon_hooks``; recreate it and
    register the ctypes-based NTFF profile hook (same as trn_boot's
    ``_ntff_profile_via_ctypes``) so ``trace=True`` yields exec_time_ns."""
    import contextlib
    import ctypes
    import types

    if "antenv.axon_hooks" in sys.modules:
        return True
    so_path = "/opt/axon/libaxon_pjrt.so"
    if not os.path.exists(so_path):
        return False
    lib = ctypes.CDLL(so_path)
    if not hasattr(lib, "axon_start_nrt_profile"):
        return False
    lib.axon_start_nrt_profile.argtypes = [
        ctypes.POINTER(ctypes.c_int64),
        ctypes.c_size_t,
    ]
    lib.axon_start_nrt_profile.restype = ctypes.c_int64
    lib.axon_stop_nrt_profile.argtypes = [ctypes.c_char_p]
    lib.axon_stop_nrt_profile.restype = ctypes.c_int64

    @contextlib.contextmanager
    def _hook(output_dir, device_ids):
        import jax

        jax.devices()
        if device_ids:
            ids = (ctypes.c_int64 * len(device_ids))(*device_ids)
            rc = lib.axon_start_nrt_profile(ids, len(device_ids))
        else:
            rc = lib.axon_start_nrt_profile(None, 0)
        if rc != 0:
            raise RuntimeError(f"axon_start_nrt_profile rc={rc}")
        try:
            yield
        finally:
            n = lib.axon_stop_nrt_profile(str(output_dir).encode())
            print(f"ntff profile: {n} file(s) written to {output_dir}")

    mod = types.ModuleType("antenv.axon_hooks")
    _state = {"hook": _hook}
    mod.set_axon_ntff_profile_hook = lambda h: _state.__setitem__("hook", h)
    mod.get_axon_ntff_profile_hook = lambda: _state["hook"]
    sys.modules["antenv.axon_hooks"] = mod
    import antenv

    antenv.axon_hooks = mod
    return True


def run(x, A, B, C, trace=False):
    """Run on hardware; returns (y_full, exec_time_ns_or_None)."""
    from concourse import bass_utils
    from concourse.bass_interp import get_hw_module

    if trace and not _install_ntff_hook():
        trace = False
    if trace:
        # upload_artifacts pushes the NEFF dir to a remote bucket; in this
        # sandbox that can fail AFTER a successful run, losing the results.
        # Degrade to the local path. (Only touches the tracing dev path.)
        if not getattr(bass_utils.upload_artifacts, "_safe", False):
            _orig_upload = bass_utils.upload_artifacts

            def _safe_upload(tmpdir):
                try:
                    return _orig_upload(tmpdir)
                except Exception as e:
                    print(f"upload_artifacts skipped ({type(e).__name__}): {e}")
                    return str(tmpdir)

            _safe_upload._safe = True
            bass_utils.upload_artifacts = _safe_upload

    nc = _get_nc()
    in_maps = _make_in_maps(x, A, B, C)

    old_m = nc.m
    nc.m = get_hw_module(nc.m)
    try:
        res = bass_utils.run_bass_kernel_spmd(
            nc, in_maps, core_ids=list(range(NCORES)), trace=trace
        )
    finally:
        nc.m = old_m

    y = np.empty((ROWS, D), dtype=np.float32)
    for c in range(NCORES):
        rg, cg = divmod(c, G)
        yp = np.asarray(res.results[c]["y_d"]).astype(np.float32)  # [W_COL, R]
        y[rg * R : (rg + 1) * R, cg * W_COL : (cg + 1) * W_COL] = yp.T
    return y.reshape(BATCH, SEQ, D), res.exec_time_ns


def kernel(x, A, B, C):
    y, _ = run(x, A, B, C, trace=False)
    return y
